# revision 6
# baseline (speedup 1.0000x reference)
"""VQ codebook (nn_Extractor) Trainium2 kernel.

Full inputs in, full outputs out. Internally: data-parallel over q's batch dim
across 8 NeuronCores, codebook replicated, ReduceScatter(sum) of the per-shard
segment sums + counts, EMA update sharded over codes (128 codes/core).

Per core pipeline:
  - distances via fp32r matmul  s[b,n] = q.g_n - ||g_n||^2/2  (argmax = nearest)
  - top-2 candidates (Max8/MaxIndex), exact fp32 rescore via indirect gather +
    row-wise dot (TensorTensorReduce) -> exact argmin + qld
  - one-hot scatter matmul in bf16 (counts via ones column)
  - ReduceScatter, EMA update on the core's code slice
"""

import os
import sys

sys.path.insert(0, "/opt/trn_rl_repo")

import numpy as np

import concourse.bass as bass
import concourse.bacc as bacc
import concourse.tile as tile
import concourse.mybir as mybir
from concourse.bass_utils import run_bass_kernel_spmd
from concourse.masks import make_identity

F32 = mybir.dt.float32
F32R = mybir.dt.float32r
BF16 = mybir.dt.bfloat16
U32 = mybir.dt.uint32
ALU = mybir.AluOpType

# Problem constants
B_FULL = 16384
N_CODES = 1024
QD = 2048          # Q_LEN * D_MODEL
D_MODEL = 256
DECAY = 0.99
EPSILON = 1e-5
N_CORES = 8
P = 128

GW = QD + 4        # padded gather-table row width (gnorm rides at col QD)


def _r11(x):
    """round-to-nearest 11 explicit mantissa bits (the fp32r storage format)"""
    xb = np.ascontiguousarray(x, np.float32).view(np.uint32)
    q = np.uint32(0xFFFFFFFF) << np.uint32(12)
    half = np.uint32(1) << np.uint32(11)
    return ((xb + half) & q).view(np.float32)


def build_kernel(m1: float, n_cores: int = N_CORES, b_shard: int = B_FULL // N_CORES,
                 n_codes: int = N_CODES):
    """m1 = n / (n + QD*EPSILON) baked in as an immediate."""
    BT = b_shard // P          # b-tiles per core
    KT = QD // P               # contraction tiles
    NT = n_codes // P          # code tiles
    RS = n_codes // n_cores    # codes per core after ReduceScatter
    RT = RS // P               # code tiles per core in the EMA phase

    NO_GATHER = os.environ.get("K_NO_GATHER") == "1"
    NO_COLLECTIVE = os.environ.get("K_NO_COLLECTIVE") == "1"
    NO_AUG = os.environ.get("K_NO_AUG") == "1"
    nc = bacc.Bacc("TRN2", target_bir_lowering=False, debug=False,
                   num_devices=n_cores)

    q_in = nc.dram_tensor("q", [b_shard, QD], F32, kind="ExternalInput").ap()
    gaug_in = nc.dram_tensor("gaug", [n_codes, GW], F32, kind="ExternalInput").ap()
    gneg2_in = nc.dram_tensor("gneg2", [2, n_codes], F32, kind="ExternalInput").ap()
    emadw_in = nc.dram_tensor("emadw", [RS, QD], F32, kind="ExternalInput").ap()
    emacnt_in = nc.dram_tensor("emacnt", [RS, 1], F32, kind="ExternalInput").ap()

    qld_out = nc.dram_tensor("qld_s", [b_shard], F32, kind="ExternalOutput").ap()
    ec_out = nc.dram_tensor("ec_s", [RS, 1], F32, kind="ExternalOutput").ap()
    edw_out = nc.dram_tensor("edw_s", [RS, QD], F32, kind="ExternalOutput").ap()
    ctx_out = nc.dram_tensor("ctx_s", [RS, QD], F32, kind="ExternalOutput").ap()

    replica = [list(range(n_cores))]

    with tile.TileContext(nc) as tc:
        with tc.tile_pool(name="const", bufs=1) as const, \
             tc.tile_pool(name="dram", bufs=1, space="DRAM") as dram:
            identity = const.tile([P, P], F32)
            make_identity(nc, identity)

            iota_f = const.tile([P, n_codes], F32)
            nc.gpsimd.iota(iota_f[:], pattern=[[1, n_codes]], base=0,
                           channel_multiplier=0,
                           allow_small_or_imprecise_dtypes=True)

            # augmentation rows: ones2 (K=2 stationary), gneg2 (hi/lo of -|g|^2/2)
            ones2_f = const.tile([2, P], F32)
            nc.vector.memset(ones2_f[:], 1.0)
            ones2 = const.tile([2, P], F32R)
            nc.vector.tensor_copy(ones2[:], ones2_f[:])

            gneg2_f = const.tile([2, n_codes], F32)
            nc.sync.dma_start(gneg2_f[:], gneg2_in[:])
            gneg2 = const.tile([2, n_codes], F32R)
            nc.vector.tensor_copy(gneg2[:], gneg2_f[:])

            ones_col = const.tile([P, 1], BF16)
            nc.vector.memset(ones_col[:], 1.0)

            # batched per-b-tile scalars
            qn_all = const.tile([P, BT], F32)
            gn_all = const.tile([P, 2 * BT], F32)    # [:, j*BT + i]
            dot_all = const.tile([P, 2 * BT], F32)   # [:, j*BT + i]
            idxf_all = const.tile([P, 2 * BT], F32)
            idxfin = const.tile([P, BT], F32)
            qld_all = const.tile([P, BT], F32)

            # DRAM bounces for the collective
            qhat_dram = dram.tile([n_codes, GW], F32)
            rs_dram = dram.tile([RS, GW], F32)

            # ---------------- phase G: transpose codebook -> gr ----------------
            grp_ctx = tc.tile_pool(name="grpool", bufs=1)
            grpool = grp_ctx.__enter__()
            gr = grpool.tile([P, KT * n_codes], F32R)  # block k at cols [k*N..]
            with tc.tile_pool(name="gprep", bufs=NT) as gprep, \
                 tc.tile_pool(name="psum_tg", bufs=2, space="PSUM") as psum_tg:
                g_nat = []
                for j in range(NT):
                    gt = gprep.tile([P, QD], F32, name=f"g_nat{j}", tag="g_nat")
                    nc.sync.dma_start(gt[:], gaug_in[j * P:(j + 1) * P, 0:QD])
                    g_nat.append(gt)
                for k in range(KT):
                    for jb in range((NT + 3) // 4):
                        nj = min(4, NT - jb * 4)
                        pst = psum_tg.tile([P, 512], F32, name="pst", tag="pst")
                        for t in range(nj):
                            j = jb * 4 + t
                            nc.tensor.transpose(
                                pst[:, t * P:(t + 1) * P],
                                g_nat[j][:, k * P:(k + 1) * P],
                                identity[:])
                        nc.scalar.copy(
                            gr[:, k * n_codes + jb * 512:
                               k * n_codes + jb * 512 + nj * P],
                            pst[:, 0:nj * P])

            # ---------------- phase 1: distances + argmin ----------------
            with tc.tile_pool(name="qpool", bufs=3) as qpool, \
                 tc.tile_pool(name="qtr", bufs=2) as qtrp, \
                 tc.tile_pool(name="spool", bufs=2) as spool, \
                 tc.tile_pool(name="ggpool", bufs=2) as ggpool, \
                 tc.tile_pool(name="scr", bufs=2) as scrp, \
                 tc.tile_pool(name="idxp", bufs=2) as idxp, \
                 tc.tile_pool(name="psum_t", bufs=2, space="PSUM") as psum_t, \
                 tc.tile_pool(name="psum_s", bufs=4, space="PSUM") as psum_s:
                for i in range(BT):
                    q_nat = qpool.tile([P, QD], F32, name="q_nat", tag="q")
                    nc.sync.dma_start(q_nat[:], q_in[i * P:(i + 1) * P, :])

                    # ||q||^2 per row
                    scr0 = scrp.tile([P, QD], F32, name="scr0", tag="scr")
                    nc.scalar.activation(
                        out=scr0[:], in_=q_nat[:],
                        func=mybir.ActivationFunctionType.Square,
                        accum_out=qn_all[:, i:i + 1])

                    # transpose q tile -> qTr (fp32r)
                    qtr = qtrp.tile([P, QD], F32R, name="qtr", tag="qtr")
                    for c in range(4):
                        pst = psum_t.tile([P, 512], F32, name="pstq", tag="pstq")
                        for t in range(4):
                            k = c * 4 + t
                            nc.tensor.transpose(
                                pst[:, t * P:(t + 1) * P],
                                q_nat[:, k * P:(k + 1) * P],
                                identity[:])
                        nc.scalar.copy(qtr[:, c * 512:(c + 1) * 512], pst[:])

                    # s = q . g - |g|^2/2  via fp32r matmuls
                    nh = n_codes // 512 if n_codes >= 512 else 1
                    nw = min(512, n_codes)
                    pss = [psum_s.tile([P, nw], F32, name=f"pss{h}", tag="pss")
                           for h in range(nh)]
                    if not NO_AUG:
                        for h in range(nh):
                            nc.tensor.matmul(
                                pss[h][:], ones2[:],
                                gneg2[:, h * nw:(h + 1) * nw],
                                start=True, stop=False)
                    for k in range(KT):
                        for h in range(nh):
                            nc.tensor.matmul(
                                pss[h][:],
                                qtr[:, k * P:(k + 1) * P],
                                gr[:, k * n_codes + h * nw:
                                   k * n_codes + (h + 1) * nw],
                                start=(NO_AUG and k == 0),
                                stop=(k == KT - 1))

                    s_sb = spool.tile([P, n_codes], F32, name="s_sb", tag="s")
                    for h in range(nh):
                        nc.scalar.copy(s_sb[:, h * nw:(h + 1) * nw], pss[h][:])

                    # top-2 candidates
                    mx8 = idxp.tile([P, 8], F32, name="mx8", tag="mx8")
                    idx8 = idxp.tile([P, 8], U32, name="idx8", tag="idx8")
                    nc.vector.max(mx8[:], s_sb[:])
                    nc.vector.max_index(idx8[:], mx8[:], s_sb[:])
                    nc.vector.tensor_copy(idxf_all[:, i:i + 1], idx8[:, 0:1])
                    nc.vector.tensor_copy(idxf_all[:, BT + i:BT + i + 1],
                                          idx8[:, 1:2])

                    if not NO_GATHER:
                        # gather both candidate rows (+their |g|^2 at col QD)
                        gg = ggpool.tile([P, 2 * GW], F32, name="gg", tag="gg")
                        for j in range(2):
                            nc.gpsimd.indirect_dma_start(
                                out=gg[:, j * GW:(j + 1) * GW],
                                out_offset=None,
                                in_=gaug_in[:],
                                in_offset=bass.IndirectOffsetOnAxis(
                                    ap=idx8[:, j:j + 1], axis=0))

                        # rescore: dot_all[:, j*BT+i] = sum(-2 q.g_c)
                        for j in range(2):
                            scr1 = scrp.tile([P, QD], F32, name="scr1",
                                             tag="scr")
                            nc.vector.scalar_tensor_tensor(
                                out=scr1[:], in0=gg[:, j * GW:j * GW + QD],
                                scalar=-2.0, in1=q_nat[:],
                                op0=ALU.mult, op1=ALU.mult,
                                accum_out=dot_all[:, j * BT + i:j * BT + i + 1])
                            nc.vector.tensor_copy(
                                gn_all[:, j * BT + i:j * BT + i + 1],
                                gg[:, j * GW + QD:j * GW + QD + 1])
                    else:
                        nc.vector.memset(dot_all[:, i:i + 1], 0.0)
                        nc.vector.memset(dot_all[:, BT + i:BT + i + 1], 1.0)
                        nc.vector.memset(gn_all[:, i:i + 1], 0.0)
                        nc.vector.memset(gn_all[:, BT + i:BT + i + 1], 0.0)

                # batched select + qld (d_j = qn + gn_j - 2 dot_j)
                d0 = const.tile([P, BT], F32)
                d1 = const.tile([P, BT], F32)
                nc.vector.tensor_tensor(out=d0[:], in0=qn_all[:],
                                        in1=gn_all[:, 0:BT], op=ALU.add)
                nc.vector.tensor_tensor(out=d0[:], in0=d0[:],
                                        in1=dot_all[:, 0:BT], op=ALU.add)
                nc.vector.tensor_tensor(out=d1[:], in0=qn_all[:],
                                        in1=gn_all[:, BT:2 * BT], op=ALU.add)
                nc.vector.tensor_tensor(out=d1[:], in0=d1[:],
                                        in1=dot_all[:, BT:2 * BT], op=ALU.add)
                cmp = const.tile([P, BT], F32)
                nc.vector.tensor_tensor(out=cmp[:], in0=d1[:], in1=d0[:],
                                        op=ALU.is_lt)
                dmin = const.tile([P, BT], F32)
                nc.vector.tensor_tensor(out=dmin[:], in0=d0[:], in1=d1[:],
                                        op=ALU.min)
                nc.vector.tensor_scalar(out=qld_all[:], in0=dmin[:],
                                        scalar1=1.0 / QD, scalar2=None,
                                        op0=ALU.mult)
                # idxfin = idx0 + cmp * (idx1 - idx0)
                idxd = const.tile([P, BT], F32)
                nc.vector.tensor_tensor(out=idxd[:], in0=idxf_all[:, BT:2 * BT],
                                        in1=idxf_all[:, 0:BT], op=ALU.subtract)
                nc.vector.tensor_tensor(out=idxd[:], in0=idxd[:], in1=cmp[:],
                                        op=ALU.mult)
                nc.vector.tensor_tensor(out=idxfin[:], in0=idxf_all[:, 0:BT],
                                        in1=idxd[:], op=ALU.add)

                # qld out: element (p, i) -> qld[i*P + p]
                nc.sync.dma_start(
                    qld_out.rearrange("(i p) -> p i", p=P), qld_all[:])
            grp_ctx.__exit__(None, None, None)

            # ---------------- phase 2: one-hot scatter matmul (bf16) -----------
            with tc.tile_pool(name="oh", bufs=BT) as ohp, \
                 tc.tile_pool(name="qb", bufs=BT) as qbp, \
                 tc.tile_pool(name="stg", bufs=6) as stgp, \
                 tc.tile_pool(name="psum_h", bufs=6, space="PSUM") as psum_h, \
                 tc.tile_pool(name="psum_c", bufs=2, space="PSUM") as psum_c:
                onehot = []
                for i in range(BT):
                    oh = ohp.tile([P, n_codes], BF16, name=f"oh{i}", tag="oh")
                    nc.vector.tensor_scalar(
                        out=oh[:], in0=iota_f[:], scalar1=idxfin[:, i:i + 1],
                        scalar2=None, op0=ALU.is_equal)
                    onehot.append(oh)
                qbs = []
                for i in range(BT):
                    qb = qbp.tile([P, QD], BF16, name=f"qb{i}", tag="qb",
                                  bufs=BT)
                    nc.gpsimd.dma_start(qb[:], q_in[i * P:(i + 1) * P, :])
                    qbs.append(qb)

                QG = QD // 512
                for n in range(NT):
                    phs = [psum_h.tile([P, 512], F32, name=f"ph{c}", tag="ph")
                           for c in range(QG)]
                    pc = psum_c.tile([P, 512], F32, name="pc", tag="pc")
                    for i in range(BT):
                        for c in range(QG):
                            nc.tensor.matmul(
                                phs[c][:],
                                onehot[i][:, n * P:(n + 1) * P],
                                qbs[i][:, c * 512:(c + 1) * 512],
                                start=(i == 0), stop=(i == BT - 1))
                        nc.tensor.matmul(
                            pc[:, 0:1],
                            onehot[i][:, n * P:(n + 1) * P],
                            ones_col[:],
                            start=(i == 0), stop=(i == BT - 1))
                    for c in range(QG):
                        st = stgp.tile([P, 512], F32, name="st", tag="st")
                        nc.scalar.copy(st[:], phs[c][:])
                        nc.sync.dma_start(
                            qhat_dram[n * P:(n + 1) * P, c * 512:(c + 1) * 512],
                            st[:])
                    stc = stgp.tile([P, GW - QD], F32, name="stc", tag="stc")
                    nc.vector.memset(stc[:], 0.0)
                    nc.scalar.copy(stc[:, 0:1], pc[:, 0:1])
                    nc.sync.dma_start(
                        qhat_dram[n * P:(n + 1) * P, QD:GW], stc[:])

            # ---------------- phase 3: ReduceScatter ----------------
            if NO_COLLECTIVE:
                nc.sync.dma_start(rs_dram[:], qhat_dram[0:RS, :])
            else:
                nc.gpsimd.collective_compute(
                    "ReduceScatter", ALU.add, replica_groups=replica,
                    ins=[qhat_dram[:].opt()], outs=[rs_dram[:].opt()])

            # ---------------- phase 4: EMA update on code slice ----------------
            with tc.tile_pool(name="ema", bufs=2) as emap:
                for r in range(RT):
                    rows = slice(r * P, (r + 1) * P)
                    qh = emap.tile([P, GW], F32, name="qh", tag="qh")
                    nc.sync.dma_start(qh[:], rs_dram[rows, :])
                    edw_t = emap.tile([P, QD], F32, name="edw_t", tag="edw")
                    nc.sync.dma_start(edw_t[:], emadw_in[rows, :])
                    ecnt = emap.tile([P, 1], F32, name="ecnt", tag="ecnt")
                    nc.sync.dma_start(ecnt[:], emacnt_in[rows, :])

                    # ec = (DECAY*cnt_old + (1-DECAY)*counts + EPS) * m1
                    ec0 = emap.tile([P, 1], F32, name="ec0", tag="ec0")
                    nc.vector.scalar_tensor_tensor(
                        out=ec0[:], in0=ecnt[:], scalar=DECAY / (1 - DECAY),
                        in1=qh[:, QD:QD + 1], op0=ALU.mult, op1=ALU.add)
                    ec1 = emap.tile([P, 1], F32, name="ec1", tag="ec1")
                    nc.vector.tensor_scalar(
                        out=ec1[:], in0=ec0[:], scalar1=(1 - DECAY),
                        scalar2=None, op0=ALU.mult)
                    ec2 = emap.tile([P, 1], F32, name="ec2", tag="ec2")
                    nc.vector.tensor_scalar(
                        out=ec2[:], in0=ec1[:], scalar1=EPSILON, scalar2=m1,
                        op0=ALU.add, op1=ALU.mult)
                    rec = emap.tile([P, 1], F32, name="rec", tag="rec")
                    nc.vector.reciprocal(rec[:], ec2[:])

                    # edw = (ema_dw * (DECAY/(1-DECAY)) + q_hat) * (1-DECAY)
                    tmp = emap.tile([P, QD], F32, name="tmp", tag="tmp")
                    nc.vector.scalar_tensor_tensor(
                        out=tmp[:], in0=edw_t[:], scalar=DECAY / (1 - DECAY),
                        in1=qh[:, 0:QD], op0=ALU.mult, op1=ALU.add)
                    edw_f = emap.tile([P, QD], F32, name="edw_f", tag="edw_f")
                    nc.vector.tensor_scalar(
                        out=edw_f[:], in0=tmp[:], scalar1=(1 - DECAY),
                        scalar2=None, op0=ALU.mult)
                    ctx_f = emap.tile([P, QD], F32, name="ctx_f", tag="ctx_f")
                    nc.vector.tensor_scalar(
                        out=ctx_f[:], in0=edw_f[:], scalar1=rec[:, 0:1],
                        scalar2=None, op0=ALU.mult)

                    nc.sync.dma_start(ec_out[rows, :], ec2[:])
                    nc.sync.dma_start(edw_out[rows, :], edw_f[:])
                    nc.sync.dma_start(ctx_out[rows, :], ctx_f[:])

    nc.compile()
    return nc


_CACHE = {}


def _make_runner(nc, n_cores):
    """Persistent jitted PJRT runner (mirrors bass2jax.run_bass_via_pjrt but
    cached so repeated kernel() calls don't re-trace/re-compile)."""
    import jax
    from jax.experimental.shard_map import shard_map
    from jax.sharding import Mesh, PartitionSpec
    from concourse import bass2jax

    bass2jax.install_neuronx_cc_hook()
    partition_name = (nc.partition_id_tensor.name
                      if nc.partition_id_tensor else None)
    in_names, out_names, out_avals = [], [], []
    for alloc in nc.m.functions[0].allocations:
        if not isinstance(alloc, mybir.MemoryLocationSet):
            continue
        name = alloc.memorylocations[0].name
        if alloc.kind == "ExternalInput":
            if name != partition_name:
                in_names.append(name)
        elif alloc.kind == "ExternalOutput":
            out_names.append(name)
            out_avals.append(jax.core.ShapedArray(
                tuple(alloc.tensor_shape), mybir.dt.np(alloc.dtype)))
    n_params = len(in_names)
    n_outs = len(out_avals)
    donate = tuple(range(n_params, n_params + n_outs))
    bind_in_names = list(in_names) + list(out_names)
    if partition_name is not None:
        bind_in_names.append(partition_name)

    def _body(*args):
        operands = list(args)
        if partition_name is not None:
            operands.append(bass2jax.partition_id_tensor())
        return tuple(bass2jax._bass_exec_p.bind(
            *operands, out_avals=tuple(out_avals),
            in_names=tuple(bind_in_names), out_names=tuple(out_names),
            lowering_input_output_aliases=(),
            sim_require_finite=True, sim_require_nnan=True, nc=nc))

    devices = jax.devices()[:n_cores]
    mesh = Mesh(np.asarray(devices), ("core",))
    in_specs = (PartitionSpec("core"),) * (n_params + n_outs)
    out_specs = (PartitionSpec("core"),) * n_outs
    fn = jax.jit(
        shard_map(_body, mesh=mesh, in_specs=in_specs, out_specs=out_specs,
                  check_rep=False),
        donate_argnums=donate, keep_unused=True)
    return {"fn": fn, "in_names": in_names, "out_names": out_names,
            "out_avals": out_avals, "mesh": mesh}


def _get_kernel(m1: float):
    key = round(float(m1), 12)
    if key not in _CACHE:
        nc = build_kernel(key)
        _CACHE[key] = _make_runner(nc, N_CORES)
    return _CACHE[key]


def _concat_inputs(runner, in_maps):
    return [np.concatenate([np.asarray(m[name]) for m in in_maps], axis=0)
            for name in runner["in_names"]]


def _zero_outs(runner, n_cores):
    return [np.zeros((n_cores * a.shape[0], *a.shape[1:]), a.dtype)
            for a in runner["out_avals"]]


def _run(runner, in_maps, n_cores):
    outs = runner["fn"](*_concat_inputs(runner, in_maps),
                        *_zero_outs(runner, n_cores))
    results = []
    for c in range(n_cores):
        results.append({
            name: np.asarray(outs[i]).reshape(
                n_cores, *runner["out_avals"][i].shape)[c]
            for i, name in enumerate(runner["out_names"])})
    return results


def kernel(q, context, ema_count, ema_dw, _want_results=False, _trace=False):
    q = np.ascontiguousarray(q, np.float32)
    context = np.ascontiguousarray(context, np.float32)
    ema_count = np.ascontiguousarray(ema_count, np.float32)
    ema_dw = np.ascontiguousarray(ema_dw, np.float32)

    B, Q, D = q.shape
    N = context.shape[0]
    q2 = q.reshape(B, Q * D)
    g2 = context.reshape(N, Q * D)
    edw2 = ema_dw.reshape(N, Q * D)

    gn64 = np.sum(g2.astype(np.float64) ** 2, axis=1)
    gn = gn64.astype(np.float32)
    gaug = np.zeros((N, GW), np.float32)
    gaug[:, :Q * D] = g2
    gaug[:, Q * D] = gn

    gneg = (-0.5 * gn64).astype(np.float32)
    gneg_hi = _r11(gneg)
    gneg_lo = _r11(gneg - gneg_hi)
    gneg2 = np.stack([gneg_hi, gneg_lo], axis=0)

    # n = sum(ec) is input-derived: sum(counts) == B exactly
    n_val = np.float32(DECAY) * np.float32(np.sum(ema_count.astype(np.float64))) \
        + np.float32(1 - DECAY) * np.float32(B)
    m1 = float(np.float64(n_val) / (np.float64(n_val) + Q * D * EPSILON))

    runner = _get_kernel(m1)

    bs = B // N_CORES
    rs = N // N_CORES
    in_maps = []
    for c in range(N_CORES):
        in_maps.append({
            "q": q2[c * bs:(c + 1) * bs],
            "gaug": gaug,
            "gneg2": gneg2,
            "emadw": edw2[c * rs:(c + 1) * rs],
            "emacnt": ema_count[c * rs:(c + 1) * rs].reshape(rs, 1),
        })

    results = _run(runner, in_maps, N_CORES)

    qld = np.concatenate([results[c]["qld_s"] for c in range(N_CORES)])
    ec = np.concatenate([results[c]["ec_s"][:, 0] for c in range(N_CORES)])
    edw = np.concatenate([results[c]["edw_s"] for c in range(N_CORES)])
    ctx = np.concatenate([results[c]["ctx_s"] for c in range(N_CORES)])

    out = (qld.astype(np.float32),
           ctx.reshape(-1, D).astype(np.float32),
           ec.astype(np.float32),
           edw.reshape(N, Q, D).astype(np.float32),
           ctx.reshape(N, Q, D).astype(np.float32))
    if _want_results:
        return out, results
    return out


# revision 7
# speedup vs baseline: 84.9896x; 84.9896x over previous
"""VQ codebook (nn_Extractor) Trainium2 kernel.

Full inputs in, full outputs out. Internally: data-parallel over q's batch dim
across 8 NeuronCores, codebook replicated, ReduceScatter(sum) of the per-shard
segment sums + counts, EMA update sharded over codes (128 codes/core).

Per core pipeline:
  - distances via fp32r matmul  s[b,n] = q.g_n - ||g_n||^2/2  (argmax = nearest)
  - top-2 candidates (Max8/MaxIndex), exact fp32 rescore via indirect gather +
    row-wise dot (TensorTensorReduce) -> exact argmin + qld
  - one-hot scatter matmul in bf16 (counts via ones column)
  - ReduceScatter, EMA update on the core's code slice
"""

import os
import sys

sys.path.insert(0, "/opt/trn_rl_repo")

import numpy as np

import concourse.bass as bass
import concourse.bacc as bacc
import concourse.tile as tile
import concourse.mybir as mybir
from concourse.bass_utils import run_bass_kernel_spmd
from concourse.masks import make_identity

F32 = mybir.dt.float32
F32R = mybir.dt.float32r
BF16 = mybir.dt.bfloat16
U32 = mybir.dt.uint32
ALU = mybir.AluOpType

# Problem constants
B_FULL = 16384
N_CODES = 1024
QD = 2048          # Q_LEN * D_MODEL
D_MODEL = 256
DECAY = 0.99
EPSILON = 1e-5
N_CORES = 8
P = 128

GW = QD + 4        # padded gather-table row width (gnorm rides at col QD)


def _r11(x):
    """round-to-nearest 11 explicit mantissa bits (the fp32r storage format)"""
    xb = np.ascontiguousarray(x, np.float32).view(np.uint32)
    q = np.uint32(0xFFFFFFFF) << np.uint32(12)
    half = np.uint32(1) << np.uint32(11)
    return ((xb + half) & q).view(np.float32)


def build_kernel(m1: float, n_cores: int = N_CORES, b_shard: int = B_FULL // N_CORES,
                 n_codes: int = N_CODES):
    """m1 = n / (n + QD*EPSILON) baked in as an immediate."""
    BT = b_shard // P          # b-tiles per core
    KT = QD // P               # contraction tiles
    NT = n_codes // P          # code tiles
    RS = n_codes // n_cores    # codes per core after ReduceScatter
    RT = RS // P               # code tiles per core in the EMA phase

    NO_GATHER = os.environ.get("K_NO_GATHER") == "1"
    NO_COLLECTIVE = os.environ.get("K_NO_COLLECTIVE") == "1"
    NO_AUG = os.environ.get("K_NO_AUG") == "1"
    nc = bacc.Bacc("TRN2", target_bir_lowering=False, debug=False,
                   num_devices=n_cores)

    q_in = nc.dram_tensor("q", [b_shard, QD], F32, kind="ExternalInput").ap()
    gaug_in = nc.dram_tensor("gaug", [n_codes, GW], F32, kind="ExternalInput").ap()
    gneg2_in = nc.dram_tensor("gneg2", [2, n_codes], F32, kind="ExternalInput").ap()
    emadw_in = nc.dram_tensor("emadw", [RS, QD], F32, kind="ExternalInput").ap()
    emacnt_in = nc.dram_tensor("emacnt", [RS, 1], F32, kind="ExternalInput").ap()

    qld_out = nc.dram_tensor("qld_s", [b_shard], F32, kind="ExternalOutput").ap()
    ec_out = nc.dram_tensor("ec_s", [RS, 1], F32, kind="ExternalOutput").ap()
    edw_out = nc.dram_tensor("edw_s", [RS, QD], F32, kind="ExternalOutput").ap()
    ctx_out = nc.dram_tensor("ctx_s", [RS, QD], F32, kind="ExternalOutput").ap()

    replica = [list(range(n_cores))]

    with tile.TileContext(nc) as tc:
        with tc.tile_pool(name="const", bufs=1) as const, \
             tc.tile_pool(name="dram", bufs=1, space="DRAM") as dram:
            identity = const.tile([P, P], F32)
            make_identity(nc, identity)

            iota_f = const.tile([P, n_codes], F32)
            nc.gpsimd.iota(iota_f[:], pattern=[[1, n_codes]], base=0,
                           channel_multiplier=0,
                           allow_small_or_imprecise_dtypes=True)

            # augmentation rows: ones2 (K=2 stationary), gneg2 (hi/lo of -|g|^2/2)
            ones2_f = const.tile([2, P], F32)
            nc.vector.memset(ones2_f[:], 1.0)
            ones2 = const.tile([2, P], F32R)
            nc.vector.tensor_copy(ones2[:], ones2_f[:])

            gneg2_f = const.tile([2, n_codes], F32)
            nc.sync.dma_start(gneg2_f[:], gneg2_in[:])
            gneg2 = const.tile([2, n_codes], F32R)
            nc.vector.tensor_copy(gneg2[:], gneg2_f[:])

            ones_col = const.tile([P, 1], BF16)
            nc.vector.memset(ones_col[:], 1.0)

            # batched per-b-tile scalars
            qn_all = const.tile([P, BT], F32)
            gn_all = const.tile([P, 2 * BT], F32)    # [:, j*BT + i]
            dot_all = const.tile([P, 2 * BT], F32)   # [:, j*BT + i]
            idxf_all = const.tile([P, 2 * BT], F32)
            idxfin = const.tile([P, BT], F32)
            qld_all = const.tile([P, BT], F32)

            # DRAM bounces for the collective
            qhat_dram = dram.tile([n_codes, GW], F32)
            rs_dram = dram.tile([RS, GW], F32)

            # ---------------- phase G: transpose codebook -> gr ----------------
            grp_ctx = tc.tile_pool(name="grpool", bufs=1)
            grpool = grp_ctx.__enter__()
            gr = grpool.tile([P, KT * n_codes], F32R)  # block k at cols [k*N..]
            with tc.tile_pool(name="gprep", bufs=NT) as gprep, \
                 tc.tile_pool(name="psum_tg", bufs=2, space="PSUM") as psum_tg:
                g_nat = []
                for j in range(NT):
                    gt = gprep.tile([P, QD], F32, name=f"g_nat{j}", tag="g_nat")
                    nc.sync.dma_start(gt[:], gaug_in[j * P:(j + 1) * P, 0:QD])
                    g_nat.append(gt)
                for k in range(KT):
                    for jb in range((NT + 3) // 4):
                        nj = min(4, NT - jb * 4)
                        pst = psum_tg.tile([P, 512], F32, name="pst", tag="pst")
                        for t in range(nj):
                            j = jb * 4 + t
                            nc.tensor.transpose(
                                pst[:, t * P:(t + 1) * P],
                                g_nat[j][:, k * P:(k + 1) * P],
                                identity[:])
                        nc.scalar.copy(
                            gr[:, k * n_codes + jb * 512:
                               k * n_codes + jb * 512 + nj * P],
                            pst[:, 0:nj * P])

            # ---------------- phase 1: distances + argmin ----------------
            with tc.tile_pool(name="qpool", bufs=3) as qpool, \
                 tc.tile_pool(name="qtr", bufs=2) as qtrp, \
                 tc.tile_pool(name="spool", bufs=2) as spool, \
                 tc.tile_pool(name="ggpool", bufs=2) as ggpool, \
                 tc.tile_pool(name="scr", bufs=2) as scrp, \
                 tc.tile_pool(name="idxp", bufs=2) as idxp, \
                 tc.tile_pool(name="psum_t", bufs=2, space="PSUM") as psum_t, \
                 tc.tile_pool(name="psum_s", bufs=4, space="PSUM") as psum_s:
                for i in range(BT):
                    q_nat = qpool.tile([P, QD], F32, name="q_nat", tag="q")
                    nc.sync.dma_start(q_nat[:], q_in[i * P:(i + 1) * P, :])

                    # ||q||^2 per row
                    scr0 = scrp.tile([P, QD], F32, name="scr0", tag="scr")
                    nc.scalar.activation(
                        out=scr0[:], in_=q_nat[:],
                        func=mybir.ActivationFunctionType.Square,
                        accum_out=qn_all[:, i:i + 1])

                    # transpose q tile -> qTr (fp32r)
                    qtr = qtrp.tile([P, QD], F32R, name="qtr", tag="qtr")
                    for c in range(4):
                        pst = psum_t.tile([P, 512], F32, name="pstq", tag="pstq")
                        for t in range(4):
                            k = c * 4 + t
                            nc.tensor.transpose(
                                pst[:, t * P:(t + 1) * P],
                                q_nat[:, k * P:(k + 1) * P],
                                identity[:])
                        nc.scalar.copy(qtr[:, c * 512:(c + 1) * 512], pst[:])

                    # s = q . g - |g|^2/2  via fp32r matmuls
                    nh = n_codes // 512 if n_codes >= 512 else 1
                    nw = min(512, n_codes)
                    pss = [psum_s.tile([P, nw], F32, name=f"pss{h}", tag="pss")
                           for h in range(nh)]
                    if not NO_AUG:
                        for h in range(nh):
                            nc.tensor.matmul(
                                pss[h][:], ones2[:],
                                gneg2[:, h * nw:(h + 1) * nw],
                                start=True, stop=False)
                    for k in range(KT):
                        for h in range(nh):
                            nc.tensor.matmul(
                                pss[h][:],
                                qtr[:, k * P:(k + 1) * P],
                                gr[:, k * n_codes + h * nw:
                                   k * n_codes + (h + 1) * nw],
                                start=(NO_AUG and k == 0),
                                stop=(k == KT - 1))

                    s_sb = spool.tile([P, n_codes], F32, name="s_sb", tag="s")
                    for h in range(nh):
                        nc.scalar.copy(s_sb[:, h * nw:(h + 1) * nw], pss[h][:])

                    # top-2 candidates
                    mx8 = idxp.tile([P, 8], F32, name="mx8", tag="mx8")
                    idx8 = idxp.tile([P, 8], U32, name="idx8", tag="idx8")
                    nc.vector.max(mx8[:], s_sb[:])
                    nc.vector.max_index(idx8[:], mx8[:], s_sb[:])
                    nc.vector.tensor_copy(idxf_all[:, i:i + 1], idx8[:, 0:1])
                    nc.vector.tensor_copy(idxf_all[:, BT + i:BT + i + 1],
                                          idx8[:, 1:2])

                    if not NO_GATHER:
                        # gather both candidate rows (+their |g|^2 at col QD)
                        gg = ggpool.tile([P, 2 * GW], F32, name="gg", tag="gg")
                        for j in range(2):
                            nc.gpsimd.indirect_dma_start(
                                out=gg[:, j * GW:(j + 1) * GW],
                                out_offset=None,
                                in_=gaug_in[:],
                                in_offset=bass.IndirectOffsetOnAxis(
                                    ap=idx8[:, j:j + 1], axis=0))

                        # rescore: dot_all[:, j*BT+i] = sum(-2 q.g_c)
                        for j in range(2):
                            scr1 = scrp.tile([P, QD], F32, name="scr1",
                                             tag="scr")
                            nc.vector.scalar_tensor_tensor(
                                out=scr1[:], in0=gg[:, j * GW:j * GW + QD],
                                scalar=-2.0, in1=q_nat[:],
                                op0=ALU.mult, op1=ALU.mult,
                                accum_out=dot_all[:, j * BT + i:j * BT + i + 1])
                            nc.vector.tensor_copy(
                                gn_all[:, j * BT + i:j * BT + i + 1],
                                gg[:, j * GW + QD:j * GW + QD + 1])
                    else:
                        nc.vector.memset(dot_all[:, i:i + 1], 0.0)
                        nc.vector.memset(dot_all[:, BT + i:BT + i + 1], 1.0)
                        nc.vector.memset(gn_all[:, i:i + 1], 0.0)
                        nc.vector.memset(gn_all[:, BT + i:BT + i + 1], 0.0)

                # batched select + qld (d_j = qn + gn_j - 2 dot_j)
                d0 = const.tile([P, BT], F32)
                d1 = const.tile([P, BT], F32)
                nc.vector.tensor_tensor(out=d0[:], in0=qn_all[:],
                                        in1=gn_all[:, 0:BT], op=ALU.add)
                nc.vector.tensor_tensor(out=d0[:], in0=d0[:],
                                        in1=dot_all[:, 0:BT], op=ALU.add)
                nc.vector.tensor_tensor(out=d1[:], in0=qn_all[:],
                                        in1=gn_all[:, BT:2 * BT], op=ALU.add)
                nc.vector.tensor_tensor(out=d1[:], in0=d1[:],
                                        in1=dot_all[:, BT:2 * BT], op=ALU.add)
                cmp = const.tile([P, BT], F32)
                nc.vector.tensor_tensor(out=cmp[:], in0=d1[:], in1=d0[:],
                                        op=ALU.is_lt)
                dmin = const.tile([P, BT], F32)
                nc.vector.tensor_tensor(out=dmin[:], in0=d0[:], in1=d1[:],
                                        op=ALU.min)
                nc.vector.tensor_scalar(out=qld_all[:], in0=dmin[:],
                                        scalar1=1.0 / QD, scalar2=None,
                                        op0=ALU.mult)
                # idxfin = idx0 + cmp * (idx1 - idx0)
                idxd = const.tile([P, BT], F32)
                nc.vector.tensor_tensor(out=idxd[:], in0=idxf_all[:, BT:2 * BT],
                                        in1=idxf_all[:, 0:BT], op=ALU.subtract)
                nc.vector.tensor_tensor(out=idxd[:], in0=idxd[:], in1=cmp[:],
                                        op=ALU.mult)
                nc.vector.tensor_tensor(out=idxfin[:], in0=idxf_all[:, 0:BT],
                                        in1=idxd[:], op=ALU.add)

                # qld out: element (p, i) -> qld[i*P + p]
                nc.sync.dma_start(
                    qld_out.rearrange("(i p) -> p i", p=P), qld_all[:])
            grp_ctx.__exit__(None, None, None)

            # ---------------- phase 2: one-hot scatter matmul (bf16) -----------
            with tc.tile_pool(name="oh", bufs=BT) as ohp, \
                 tc.tile_pool(name="qb", bufs=BT) as qbp, \
                 tc.tile_pool(name="stg", bufs=6) as stgp, \
                 tc.tile_pool(name="psum_h", bufs=6, space="PSUM") as psum_h, \
                 tc.tile_pool(name="psum_c", bufs=2, space="PSUM") as psum_c:
                onehot = []
                for i in range(BT):
                    oh = ohp.tile([P, n_codes], BF16, name=f"oh{i}", tag="oh")
                    nc.vector.tensor_scalar(
                        out=oh[:], in0=iota_f[:], scalar1=idxfin[:, i:i + 1],
                        scalar2=None, op0=ALU.is_equal)
                    onehot.append(oh)
                qbs = []
                for i in range(BT):
                    qb = qbp.tile([P, QD], BF16, name=f"qb{i}", tag="qb",
                                  bufs=BT)
                    nc.gpsimd.dma_start(qb[:], q_in[i * P:(i + 1) * P, :])
                    qbs.append(qb)

                QG = QD // 512
                for n in range(NT):
                    phs = [psum_h.tile([P, 512], F32, name=f"ph{c}", tag="ph")
                           for c in range(QG)]
                    pc = psum_c.tile([P, 512], F32, name="pc", tag="pc")
                    for i in range(BT):
                        for c in range(QG):
                            nc.tensor.matmul(
                                phs[c][:],
                                onehot[i][:, n * P:(n + 1) * P],
                                qbs[i][:, c * 512:(c + 1) * 512],
                                start=(i == 0), stop=(i == BT - 1))
                        nc.tensor.matmul(
                            pc[:, 0:1],
                            onehot[i][:, n * P:(n + 1) * P],
                            ones_col[:],
                            start=(i == 0), stop=(i == BT - 1))
                    for c in range(QG):
                        st = stgp.tile([P, 512], F32, name="st", tag="st")
                        nc.scalar.copy(st[:], phs[c][:])
                        nc.sync.dma_start(
                            qhat_dram[n * P:(n + 1) * P, c * 512:(c + 1) * 512],
                            st[:])
                    stc = stgp.tile([P, GW - QD], F32, name="stc", tag="stc")
                    nc.vector.memset(stc[:], 0.0)
                    nc.scalar.copy(stc[:, 0:1], pc[:, 0:1])
                    nc.sync.dma_start(
                        qhat_dram[n * P:(n + 1) * P, QD:GW], stc[:])

            # ---------------- phase 3: ReduceScatter ----------------
            if NO_COLLECTIVE:
                nc.sync.dma_start(rs_dram[:], qhat_dram[0:RS, :])
            else:
                nc.gpsimd.collective_compute(
                    "ReduceScatter", ALU.add, replica_groups=replica,
                    ins=[qhat_dram[:].opt()], outs=[rs_dram[:].opt()])

            # ---------------- phase 4: EMA update on code slice ----------------
            with tc.tile_pool(name="ema", bufs=2) as emap:
                for r in range(RT):
                    rows = slice(r * P, (r + 1) * P)
                    qh = emap.tile([P, GW], F32, name="qh", tag="qh")
                    nc.sync.dma_start(qh[:], rs_dram[rows, :])
                    edw_t = emap.tile([P, QD], F32, name="edw_t", tag="edw")
                    nc.sync.dma_start(edw_t[:], emadw_in[rows, :])
                    ecnt = emap.tile([P, 1], F32, name="ecnt", tag="ecnt")
                    nc.sync.dma_start(ecnt[:], emacnt_in[rows, :])

                    # ec = (DECAY*cnt_old + (1-DECAY)*counts + EPS) * m1
                    ec0 = emap.tile([P, 1], F32, name="ec0", tag="ec0")
                    nc.vector.scalar_tensor_tensor(
                        out=ec0[:], in0=ecnt[:], scalar=DECAY / (1 - DECAY),
                        in1=qh[:, QD:QD + 1], op0=ALU.mult, op1=ALU.add)
                    ec1 = emap.tile([P, 1], F32, name="ec1", tag="ec1")
                    nc.vector.tensor_scalar(
                        out=ec1[:], in0=ec0[:], scalar1=(1 - DECAY),
                        scalar2=None, op0=ALU.mult)
                    ec2 = emap.tile([P, 1], F32, name="ec2", tag="ec2")
                    nc.vector.tensor_scalar(
                        out=ec2[:], in0=ec1[:], scalar1=EPSILON, scalar2=m1,
                        op0=ALU.add, op1=ALU.mult)
                    rec = emap.tile([P, 1], F32, name="rec", tag="rec")
                    nc.vector.reciprocal(rec[:], ec2[:])

                    # edw = (ema_dw * (DECAY/(1-DECAY)) + q_hat) * (1-DECAY)
                    tmp = emap.tile([P, QD], F32, name="tmp", tag="tmp")
                    nc.vector.scalar_tensor_tensor(
                        out=tmp[:], in0=edw_t[:], scalar=DECAY / (1 - DECAY),
                        in1=qh[:, 0:QD], op0=ALU.mult, op1=ALU.add)
                    edw_f = emap.tile([P, QD], F32, name="edw_f", tag="edw_f")
                    nc.vector.tensor_scalar(
                        out=edw_f[:], in0=tmp[:], scalar1=(1 - DECAY),
                        scalar2=None, op0=ALU.mult)
                    ctx_f = emap.tile([P, QD], F32, name="ctx_f", tag="ctx_f")
                    nc.vector.tensor_scalar(
                        out=ctx_f[:], in0=edw_f[:], scalar1=rec[:, 0:1],
                        scalar2=None, op0=ALU.mult)

                    nc.sync.dma_start(ec_out[rows, :], ec2[:])
                    nc.sync.dma_start(edw_out[rows, :], edw_f[:])
                    nc.sync.dma_start(ctx_out[rows, :], ctx_f[:])

    nc.compile()
    return nc


_CACHE = {}
_NC_CACHE = {}


def _make_runner(nc, n_cores, repeat=1):
    """Persistent jitted PJRT runner (mirrors bass2jax.run_bass_via_pjrt but
    cached so repeated kernel() calls don't re-trace/re-compile).
    repeat>1 chains that many NEFF executions inside one jit call (outputs
    feed the next execution's donated buffers) for differential timing."""
    import jax
    from jax.experimental.shard_map import shard_map
    from jax.sharding import Mesh, PartitionSpec
    from concourse import bass2jax

    bass2jax.install_neuronx_cc_hook()
    partition_name = (nc.partition_id_tensor.name
                      if nc.partition_id_tensor else None)
    in_names, out_names, out_avals = [], [], []
    for alloc in nc.m.functions[0].allocations:
        if not isinstance(alloc, mybir.MemoryLocationSet):
            continue
        name = alloc.memorylocations[0].name
        if alloc.kind == "ExternalInput":
            if name != partition_name:
                in_names.append(name)
        elif alloc.kind == "ExternalOutput":
            out_names.append(name)
            out_avals.append(jax.core.ShapedArray(
                tuple(alloc.tensor_shape), mybir.dt.np(alloc.dtype)))
    n_params = len(in_names)
    n_outs = len(out_avals)
    donate = tuple(range(n_params, n_params + n_outs))
    bind_in_names = list(in_names) + list(out_names)
    if partition_name is not None:
        bind_in_names.append(partition_name)

    def _body(*args):
        params = list(args[:n_params])
        bufs = list(args[n_params:])
        for _ in range(repeat):
            operands = params + bufs
            if partition_name is not None:
                operands.append(bass2jax.partition_id_tensor())
            bufs = list(bass2jax._bass_exec_p.bind(
                *operands, out_avals=tuple(out_avals),
                in_names=tuple(bind_in_names), out_names=tuple(out_names),
                lowering_input_output_aliases=(),
                sim_require_finite=True, sim_require_nnan=True, nc=nc))
        return tuple(bufs)

    devices = jax.devices()[:n_cores]
    mesh = Mesh(np.asarray(devices), ("core",))
    in_specs = (PartitionSpec("core"),) * (n_params + n_outs)
    out_specs = (PartitionSpec("core"),) * n_outs
    fn = jax.jit(
        shard_map(_body, mesh=mesh, in_specs=in_specs, out_specs=out_specs,
                  check_rep=False),
        donate_argnums=donate, keep_unused=True)
    return {"fn": fn, "in_names": in_names, "out_names": out_names,
            "out_avals": out_avals, "mesh": mesh}


def _get_kernel(m1: float):
    key = round(float(m1), 12)
    if key not in _CACHE:
        nc = build_kernel(key)
        _NC_CACHE[key] = nc
        _CACHE[key] = _make_runner(nc, N_CORES)
    return _CACHE[key]


def _concat_inputs(runner, in_maps):
    return [np.concatenate([np.asarray(m[name]) for m in in_maps], axis=0)
            for name in runner["in_names"]]


def _zero_outs(runner, n_cores):
    return [np.zeros((n_cores * a.shape[0], *a.shape[1:]), a.dtype)
            for a in runner["out_avals"]]


def _run(runner, in_maps, n_cores):
    outs = runner["fn"](*_concat_inputs(runner, in_maps),
                        *_zero_outs(runner, n_cores))
    results = []
    for c in range(n_cores):
        results.append({
            name: np.asarray(outs[i]).reshape(
                n_cores, *runner["out_avals"][i].shape)[c]
            for i, name in enumerate(runner["out_names"])})
    return results


def kernel(q, context, ema_count, ema_dw, _want_results=False, _trace=False):
    q = np.ascontiguousarray(q, np.float32)
    context = np.ascontiguousarray(context, np.float32)
    ema_count = np.ascontiguousarray(ema_count, np.float32)
    ema_dw = np.ascontiguousarray(ema_dw, np.float32)

    B, Q, D = q.shape
    N = context.shape[0]
    q2 = q.reshape(B, Q * D)
    g2 = context.reshape(N, Q * D)
    edw2 = ema_dw.reshape(N, Q * D)

    gn64 = np.sum(g2.astype(np.float64) ** 2, axis=1)
    gn = gn64.astype(np.float32)
    gaug = np.zeros((N, GW), np.float32)
    gaug[:, :Q * D] = g2
    gaug[:, Q * D] = gn

    gneg = (-0.5 * gn64).astype(np.float32)
    gneg_hi = _r11(gneg)
    gneg_lo = _r11(gneg - gneg_hi)
    gneg2 = np.stack([gneg_hi, gneg_lo], axis=0)

    # n = sum(ec) is input-derived: sum(counts) == B exactly
    n_val = np.float32(DECAY) * np.float32(np.sum(ema_count.astype(np.float64))) \
        + np.float32(1 - DECAY) * np.float32(B)
    m1 = float(np.float64(n_val) / (np.float64(n_val) + Q * D * EPSILON))

    runner = _get_kernel(m1)

    bs = B // N_CORES
    rs = N // N_CORES
    in_maps = []
    for c in range(N_CORES):
        in_maps.append({
            "q": q2[c * bs:(c + 1) * bs],
            "gaug": gaug,
            "gneg2": gneg2,
            "emadw": edw2[c * rs:(c + 1) * rs],
            "emacnt": ema_count[c * rs:(c + 1) * rs].reshape(rs, 1),
        })

    results = _run(runner, in_maps, N_CORES)

    qld = np.concatenate([results[c]["qld_s"] for c in range(N_CORES)])
    ec = np.concatenate([results[c]["ec_s"][:, 0] for c in range(N_CORES)])
    edw = np.concatenate([results[c]["edw_s"] for c in range(N_CORES)])
    ctx = np.concatenate([results[c]["ctx_s"] for c in range(N_CORES)])

    out = (qld.astype(np.float32),
           ctx.reshape(-1, D).astype(np.float32),
           ec.astype(np.float32),
           edw.reshape(N, Q, D).astype(np.float32),
           ctx.reshape(N, Q, D).astype(np.float32))
    if _want_results:
        return out, results
    return out
